# revision 30
# baseline (speedup 1.0000x reference)
"""Trainium2 Bass kernel for 3D cosine attention (nn_Attention3D).

Shards batch*heads (2*4 = 8) across the 8 NeuronCores; each core computes
full attention for one (batch, head) pair in a transposed layout, and the
host sums per-head output-projection partials.

Self-contained: hardcodes all shapes; only needs numpy + concourse (on
PYTHONPATH in this environment).
"""

import numpy as np

import concourse.bacc as bacc
import concourse.bass as bass
import concourse.tile as tile
from concourse import mybir
from concourse.bass_utils import run_bass_kernel_spmd
from concourse.tile_rust import add_dep_helper

F32 = mybir.dt.float32
F32R = mybir.dt.float32r
BF16 = mybir.dt.bfloat16
AF = mybir.ActivationFunctionType

HEADS = 4
DIM_HEAD = 32
SCALE = 10.0
B = 2
N = 4096          # D*H*W = 8*16*32
C = 128
N_CORES = 8
IC = 512          # i-chunk (query) width
N_ICH = N // IC   # 8 i-chunks
NJT = N // 128    # 32 j-tiles (key tiles of 128)
PCH = 512         # phase-1 projection chunk
N_PCH = N // PCH

# single input blob: one DMA -> one semaphore for all inputs
#   cols [0, 4096)      xT       [128, 4096]
#   cols [4096, 4160)   w_qk     [128, 64]
#   cols [4160, 4192)   w_v      [128, 32]
#   cols [4192, 4320)   w_o      [32, 128]   (rows 0-31)
#   cols [4320, 4384)   bd64     [64, 64]    (rows 0-63, block-diag ones)
#   cols [4384, 4416)   ones     [128, 32]   (all 1.0; memset can't write f32r)
#   cols [4416, 4448)   zones    [128, 32]   (col 0 = 1.0, rest 0)
BLOB_W = 4448
O_WQK = 4096
O_WV = 4160
O_WO = 4192
O_BD = 4320
O_ONES = 4384
O_ZONES = 4416


def _emit(nc, reps=1, u_dt=F32R, s_pack=2, exp_split=1):
    import contextlib
    blob = nc.dram_tensor("blob", [128, BLOB_W], F32R, kind="ExternalInput")
    outT = nc.dram_tensor("outT", [128, N], F32, kind="ExternalOutput")

    with tile.TileContext(nc) as tc:
        with (
            tc.tile_pool(name="consts", bufs=1) as consts,
            tc.tile_pool(name="work", bufs=3) as work,
            tc.tile_pool(name="upool", bufs=2) as upool,
            tc.tile_pool(name="epi", bufs=2) as epi,
            tc.tile_pool(name="ps_qk", bufs=2, space="PSUM") as ps_qk_pool,
            tc.tile_pool(name="ps_n", bufs=1, space="PSUM") as ps_n_pool,
            tc.tile_pool(name="ps_s", bufs=2, space="PSUM") as ps_s_pool,
            tc.tile_pool(name="ps_p", bufs=1, space="PSUM") as ps_p_pool,
        ):
          with (tc.For_i(0, reps, 1) if reps > 1 else contextlib.nullcontext()):
            # ---- inputs / constants to SBUF ----
            sb_blob = consts.tile([128, BLOB_W], F32R, tag="blob")
            blob_dma = nc.sync.dma_start(out=sb_blob[:], in_=blob[:, :])
            sb_xT = sb_blob[:, 0:N]
            sb_wqk = sb_blob[:, O_WQK:O_WQK + 64]
            sb_wv = sb_blob[:, O_WV:O_WV + 32]
            sb_wo = sb_blob[0:32, O_WO:O_WO + 128]
            sb_bd = sb_blob[0:64, O_BD:O_BD + 64]

            sb_ones = sb_blob[:, O_ONES:O_ONES + 32]
            sb_eps = consts.tile([64, 1], F32, tag="eps")
            nc.vector.memset(sb_eps[:], 1e-24)

            sb_qkn = consts.tile([64, N], F32R, tag="qkn")
            sb_kn0 = consts.tile([32, N], F32R, tag="kn0")
            sb_q32 = None
            if s_pack == 2:
                sb_q32 = consts.tile([64, N], F32R, tag="q32")
            # v with a ones column at index 32 -> PV matmul also yields Z
            sb_v1 = consts.tile([128, NJT, 33], u_dt, tag="v1")
            nc.vector.tensor_copy(sb_v1[:, :, 32], sb_ones[:, 0:NJT])

            # ---- phase 1: q/k projection + L2 normalization ----
            # q rows 0-31, k rows 32-63 throughout.
            for c in range(N_PCH):
                sl = slice(c * PCH, (c + 1) * PCH)
                ps_qk = ps_qk_pool.tile([64, PCH], F32, tag="qk")
                nc.tensor.matmul(
                    ps_qk[:], sb_wqk, sb_xT[:, sl], start=True, stop=True
                )
                sb_sq = work.tile([64, PCH], F32R, tag="sq")
                nc.scalar.activation(out=sb_sq[:], in_=ps_qk[:], func=AF.Square)
                ps_n = ps_n_pool.tile([64, PCH], F32, tag="n")
                nc.tensor.matmul(
                    ps_n[:], sb_bd, sb_sq[:], start=True, stop=True
                )
                # 1/sqrt(n2 + eps) = exp(-0.5 * ln(n2 + eps)); same ACT
                # table set as the softmax exp, and far more accurate than
                # the Sqrt spline.
                sb_ln = work.tile([64, PCH], F32, tag="ln")
                nc.scalar.activation(
                    out=sb_ln[:], in_=ps_n[:], func=AF.Ln, bias=sb_eps[:]
                )
                sb_inv = work.tile([64, PCH], F32, tag="inv")
                nc.scalar.activation(
                    out=sb_inv[:], in_=sb_ln[:], func=AF.Exp, scale=-0.5
                )
                nc.vector.tensor_mul(sb_qkn[:, sl], ps_qk[:], sb_inv[:])
                # relocate this chunk's kn to base partition 0 (matmul lhsT
                # and rhs must share base partition; only DMA moves rows)
                nc.sync.dma_start(out=sb_kn0[:, sl], in_=sb_qkn[32:64, sl])
                if s_pack == 2:
                    # replica of qn at base partition 32 for row-group 1
                    nc.sync.dma_start(out=sb_q32[32:64, sl], in_=sb_qkn[0:32, sl])

            # ---- phase 1b: v projection (natural [n, d] layout) ----
            for jt in range(NJT):
                ps_v = ps_qk_pool.tile([128, 32], F32, tag="qk")
                nc.tensor.matmul(
                    ps_v[:],
                    sb_xT[:, jt * 128:(jt + 1) * 128],
                    sb_wv,
                    start=True, stop=True,
                )
                nc.vector.tensor_copy(sb_v1[:, jt, 0:32], ps_v[:])

            # ---- phase 2: attention, i-chunk outer / j-tile inner ----
            # The epilogue of chunk ic is emitted after the first two
            # j-groups of chunk ic+1 so its matmuls (which wait on the DVE
            # reciprocal chain) don't stall the in-order PE queue ahead of
            # the next chunk's S matmuls.
            sb_qn = sb_qkn[0:32, :]
            sb_kn = sb_kn0
            pending_epi = None

            def emit_epilogue(ps_p, ic):
                sb_p = epi.tile([33, IC], F32, tag="p")
                nc.vector.tensor_copy(sb_p[:], ps_p[:])
                sb_rz = epi.tile([33, IC], F32, tag="rz")
                nc.vector.reciprocal(sb_rz[32:33, :], sb_p[32:33, :])
                sb_rzr = epi.tile([33, IC], F32R, tag="rzr")
                nc.vector.tensor_copy(sb_rzr[32:33, :], sb_rz[32:33, :])
                ps_bc = ps_qk_pool.tile([32, IC], F32, tag="qk")
                nc.tensor.matmul(
                    ps_bc[:], sb_ones[32:33, :], sb_rzr[32:33, :],
                    start=True, stop=True,
                )
                sb_o = epi.tile([32, IC], F32R, tag="o")
                nc.vector.tensor_mul(sb_o[:], sb_p[0:32, :], ps_bc[:])
                ps_out = ps_qk_pool.tile([128, IC], F32, tag="qk")
                nc.tensor.matmul(
                    ps_out[:], sb_wo, sb_o[:], start=True, stop=True
                )
                sb_out = epi.tile([128, IC], F32, tag="out")
                nc.vector.tensor_copy(sb_out[:], ps_out[:])
                nc.sync.dma_start(
                    out=outT[:, ic * IC:(ic + 1) * IC], in_=sb_out[:]
                )

            for ic in range(N_ICH):
                qs = sb_qn[:, ic * IC:(ic + 1) * IC]
                ps_p = ps_p_pool.tile([33, IC], F32, tag="p")
                for jg in range(NJT // 2):
                    ps_s = ps_s_pool.tile([128, 2 * IC], F32, tag="s")
                    if s_pack == 2:
                        jt0, jt1 = 2 * jg, 2 * jg + 1
                        nc.tensor.matmul(
                            ps_s[:, 0:IC],
                            sb_kn[:, jt0 * 128:(jt0 + 1) * 128],
                            qs,
                            start=True, stop=True,
                        )
                        nc.tensor.matmul(
                            ps_s[:, IC:2 * IC],
                            sb_qkn[32:64, jt1 * 128:(jt1 + 1) * 128],
                            sb_q32[32:64, ic * IC:(ic + 1) * IC],
                            start=True, stop=True,
                        )
                    else:
                        for g in range(2):
                            jt = 2 * jg + g
                            nc.tensor.matmul(
                                ps_s[:, g * IC:(g + 1) * IC],
                                sb_kn[:, jt * 128:(jt + 1) * 128],
                                qs,
                                start=True, stop=True,
                            )
                    sb_u = upool.tile([128, 2 * IC], u_dt, tag="u")
                    w = 2 * IC // exp_split
                    for e in range(exp_split):
                        nc.scalar.activation(
                            out=sb_u[:, e * w:(e + 1) * w],
                            in_=ps_s[:, e * w:(e + 1) * w],
                            func=AF.Exp, scale=SCALE,
                        )
                    for g in range(2):
                        jt = 2 * jg + g
                        nc.tensor.matmul(
                            ps_p[:],
                            sb_v1[:, jt, :],
                            sb_u[:, g * IC:(g + 1) * IC],
                            start=(jg == 0 and g == 0),
                            stop=(jg == NJT // 2 - 1 and g == 1),
                        )
                    if jg == 1 and pending_epi is not None:
                        emit_epilogue(*pending_epi)
                        pending_epi = None
                pending_epi = (ps_p, ic)
            emit_epilogue(*pending_epi)
    return nc


_NC = None

# act_info.json index of "natural_log_exp_and_others": covers both Ln and
# Exp, so one table load serves the whole kernel. The default chooser
# alternates natural_log <-> exp_and_others (14 loads x ~2.7us of pure ACT
# serialization).
_JOINT_ACT_SET = 6


def _dedup_act_table_loads(nc):
    for blk in nc.m.functions[0].blocks:
        insts = list(blk.instructions)
        keep = []
        seen_load = False
        for inst in insts:
            if type(inst).__name__ == "InstLoadActFuncSet":
                if seen_load:
                    continue
                inst.act_func_set_id = _JOINT_ACT_SET
                seen_load = True
            keep.append(inst)
        if seen_load and len(keep) != len(insts):
            blk.instructions = keep


def get_nc(reps=1):
    global _NC
    if reps == 1 and _NC is not None:
        return _NC
    nc = bacc.Bacc(trn_type="TRN2")
    _emit(nc, reps=reps)
    nc.compile()
    _dedup_act_table_loads(nc)
    if reps == 1:
        _NC = nc
    return _NC if reps == 1 else nc


LAST_RESULT = None


def round_f32r(a):
    """Round fp32 to fp32r (11-bit mantissa, low 12 bits zero), ties-to-even."""
    a = np.ascontiguousarray(a, np.float32)
    u = a.view(np.uint32).copy()
    u += 0x7FF + ((u >> 12) & 1)
    u &= np.uint32(0xFFFFF000)
    return u.view(np.float32)


def make_in_maps(x, w_qkv, w_out):
    xf = np.ascontiguousarray(np.asarray(x, np.float32)).reshape(B, N, C)
    w_qkv = np.asarray(w_qkv, np.float32)
    w_out = np.asarray(w_out, np.float32)
    in_maps = []
    for core in range(N_CORES):
        b, h = divmod(core, HEADS)
        blob = np.zeros((128, BLOB_W), np.float32)
        blob[:, 0:N] = xf[b].T
        blob[:, O_WQK:O_WQK + 32] = w_qkv[:, h * 32:(h + 1) * 32]
        blob[:, O_WQK + 32:O_WQK + 64] = w_qkv[:, 128 + h * 32:128 + (h + 1) * 32]
        blob[:, O_WV:O_WV + 32] = w_qkv[:, 256 + h * 32:256 + (h + 1) * 32]
        blob[0:32, O_WO:O_WO + 128] = w_out[h * 32:(h + 1) * 32, :]
        blob[64:96, O_WO:O_WO + 128] = w_out[h * 32:(h + 1) * 32, :]
        blob[0:32, O_BD:O_BD + 32] = 1.0
        blob[32:64, O_BD + 32:O_BD + 64] = 1.0
        blob[:, O_ONES:O_ONES + 32] = 1.0
        blob[:, O_ZONES] = 1.0
        in_maps.append({"blob": round_f32r(blob)})
    return in_maps


def kernel(x, w_qkv, w_out, b_out):
    global LAST_RESULT
    b_out = np.asarray(b_out, np.float32)
    in_maps = make_in_maps(x, w_qkv, w_out)
    res = run_bass_kernel_spmd(get_nc(), in_maps, core_ids=list(range(N_CORES)))
    LAST_RESULT = res
    out = np.zeros((B, N, C), np.float32)
    for core in range(N_CORES):
        b = core // HEADS
        out[b] += res.results[core]["outT"].T
    out += b_out[None, None, :]
    return out.reshape(B, 8, 16, 32, C)


def _emit_v3(nc, reps=1, u_dt=F32R, exp_split=1):
    """Phase-2 v3: 4x row-packed S (row groups 0-3 = two j-tiles x two
    i-chunks) and 2x col-packed PV (chunk A -> psum partitions 0-32, chunk
    B -> 64-96), dual epilogue. PSUM accumulation avoids start=True inside
    shared banks (a start clears the whole bank): explicit memset +
    has_written overwrite semantics instead."""
    import contextlib
    blob = nc.dram_tensor("blob", [128, BLOB_W], F32R, kind="ExternalInput")
    outT = nc.dram_tensor("outT", [128, N], F32, kind="ExternalOutput")

    with tile.TileContext(nc) as tc:
        with (
            tc.tile_pool(name="consts", bufs=1) as consts,
            tc.tile_pool(name="work", bufs=3) as work,
            tc.tile_pool(name="upool", bufs=3) as upool,
            tc.tile_pool(name="epi", bufs=2) as epi,
            tc.tile_pool(name="ps_s", bufs=3, space="PSUM") as ps_s_pool,
            tc.tile_pool(name="ps_p", bufs=2, space="PSUM") as ps_p_pool,
        ):
          with (tc.For_i(0, reps, 1) if reps > 1 else contextlib.nullcontext()):
            sb_blob = consts.tile([128, BLOB_W], F32R, tag="blob")
            nc.sync.dma_start(out=sb_blob[:], in_=blob[:, :])
            sb_xT = sb_blob[:, 0:N]
            sb_wqk = sb_blob[:, O_WQK:O_WQK + 64]
            sb_wv = sb_blob[:, O_WV:O_WV + 32]
            sb_wo = sb_blob[0:32, O_WO:O_WO + 128]
            sb_wo64 = sb_blob[64:96, O_WO:O_WO + 128]
            sb_bd = sb_blob[0:64, O_BD:O_BD + 64]
            sb_ones = sb_blob[:, O_ONES:O_ONES + 32]

            sb_eps = consts.tile([64, 1], F32, tag="eps")
            nc.vector.memset(sb_eps[:], 1e-24)

            sb_q4 = consts.tile([128, N], F32R, tag="q4")
            sb_k4 = consts.tile([128, N], F32R, tag="k4")
            sb_v1 = consts.tile([128, NJT, 33], u_dt, tag="v1")
            nc.vector.tensor_copy(sb_v1[:, :, 32], sb_ones[:, 0:NJT])

            # ---- phase 1 ----
            for c in range(N_PCH):
                sl = slice(c * PCH, (c + 1) * PCH)
                ps_qk = ps_s_pool.tile([64, PCH], F32, tag="s")
                nc.tensor.matmul(
                    ps_qk[:], sb_wqk, sb_xT[:, sl], start=True, stop=True
                )
                sb_sq = work.tile([64, PCH], F32R, tag="sq")
                nc.scalar.activation(out=sb_sq[:], in_=ps_qk[:], func=AF.Square)
                ps_n = ps_s_pool.tile([64, PCH], F32, tag="s")
                nc.tensor.matmul(ps_n[:], sb_bd, sb_sq[:], start=True, stop=True)
                sb_ln = work.tile([64, PCH], F32, tag="ln")
                nc.scalar.activation(
                    out=sb_ln[:], in_=ps_n[:], func=AF.Ln, bias=sb_eps[:]
                )
                sb_inv = work.tile([64, PCH], F32, tag="inv")
                nc.scalar.activation(
                    out=sb_inv[:], in_=sb_ln[:], func=AF.Exp, scale=-0.5
                )
                nc.vector.tensor_mul(
                    sb_q4[0:32, sl], ps_qk[0:32, :], sb_inv[0:32, :]
                )
                nc.vector.tensor_mul(
                    sb_k4[32:64, sl], ps_qk[32:64, :], sb_inv[32:64, :]
                )
                for base in (32, 64, 96):
                    nc.sync.dma_start(
                        out=sb_q4[base:base + 32, sl], in_=sb_q4[0:32, sl]
                    )
                for base in (0, 64, 96):
                    nc.sync.dma_start(
                        out=sb_k4[base:base + 32, sl], in_=sb_k4[32:64, sl]
                    )

            # ---- phase 1b ----
            for jt in range(NJT):
                ps_v = ps_s_pool.tile([128, 32], F32, tag="s")
                nc.tensor.matmul(
                    ps_v[:], sb_xT[:, jt * 128:(jt + 1) * 128], sb_wv,
                    start=True, stop=True,
                )
                nc.vector.tensor_copy(sb_v1[:, jt, 0:32], ps_v[:])

            # ---- phase 2 ----
            def jsl(jt):
                return slice(jt * 128, (jt + 1) * 128)

            def emit_epilogue(ps_p, ic):
                sb_p = epi.tile([33, IC], F32, tag="p")
                nc.vector.tensor_copy(sb_p[:], ps_p[:])
                sb_rz = epi.tile([33, IC], F32, tag="rz")
                nc.vector.reciprocal(sb_rz[32:33, :], sb_p[32:33, :])
                sb_rzr = epi.tile([33, IC], F32R, tag="rzr")
                nc.vector.tensor_copy(sb_rzr[32:33, :], sb_rz[32:33, :])
                ps_bc = ps_s_pool.tile([32, IC], F32, tag="s")
                nc.tensor.matmul(
                    ps_bc[:], sb_ones[32:33, :], sb_rzr[32:33, :],
                    start=True, stop=True,
                )
                sb_o = epi.tile([32, IC], F32R, tag="o")
                nc.vector.tensor_mul(sb_o[:], sb_p[0:32, :], ps_bc[:])
                ps_out = ps_s_pool.tile([128, IC], F32, tag="s")
                nc.tensor.matmul(
                    ps_out[:], sb_wo, sb_o[:], start=True, stop=True
                )
                sb_out = epi.tile([128, IC], F32, tag="out")
                nc.vector.tensor_copy(sb_out[:], ps_out[:])
                nc.sync.dma_start(
                    out=outT[:, ic * IC:(ic + 1) * IC], in_=sb_out[:]
                )

            pending_epi = []
            for icp in range(N_ICH // 2):
                icA, icB = 2 * icp, 2 * icp + 1
                qA = slice(icA * IC, (icA + 1) * IC)
                qB = slice(icB * IC, (icB + 1) * IC)
                ps_pA = ps_p_pool.tile([33, IC], F32, tag="p")
                ps_pB = ps_p_pool.tile([33, IC], F32, tag="p")
                for jg in range(NJT // 2):
                    jt0, jt1 = 2 * jg, 2 * jg + 1
                    ps_sA = ps_s_pool.tile([128, 2 * IC], F32, tag="s")
                    ps_sB = ps_s_pool.tile([128, 2 * IC], F32, tag="s")
                    nc.tensor.matmul(
                        ps_sA[:, 0:IC], sb_k4[0:32, jsl(jt0)], sb_q4[0:32, qA],
                        start=True, stop=True, tile_position=(0, 0),
                    )
                    nc.tensor.matmul(
                        ps_sA[:, IC:2 * IC], sb_k4[32:64, jsl(jt1)],
                        sb_q4[32:64, qA],
                        start=True, stop=True, tile_position=(32, 0),
                    )
                    nc.tensor.matmul(
                        ps_sB[:, 0:IC], sb_k4[64:96, jsl(jt0)],
                        sb_q4[64:96, qB],
                        start=True, stop=True, tile_position=(64, 0),
                    )
                    nc.tensor.matmul(
                        ps_sB[:, IC:2 * IC], sb_k4[96:128, jsl(jt1)],
                        sb_q4[96:128, qB],
                        start=True, stop=True, tile_position=(96, 0),
                    )
                    sb_uA = upool.tile([128, 2 * IC], u_dt, tag="u")
                    sb_uB = upool.tile([128, 2 * IC], u_dt, tag="u")
                    w = 2 * IC // exp_split
                    for sb_u, ps_s in ((sb_uA, ps_sA), (sb_uB, ps_sB)):
                        for e in range(exp_split):
                            nc.scalar.activation(
                                out=sb_u[:, e * w:(e + 1) * w],
                                in_=ps_s[:, e * w:(e + 1) * w],
                                func=AF.Exp, scale=SCALE,
                            )
                    last = (jg == NJT // 2 - 1)
                    for jt, usl in ((jt0, slice(0, IC)), (jt1, slice(IC, 2 * IC))):
                        first = (jg == 0 and jt == jt0)
                        stop = (last and jt == jt1)
                        nc.tensor.matmul(
                            ps_pA[:], sb_v1[:, jt, :], sb_uA[:, usl],
                            start=first, stop=stop,
                        )
                        nc.tensor.matmul(
                            ps_pB[:], sb_v1[:, jt, :], sb_uB[:, usl],
                            start=first, stop=stop,
                        )
                    if jg == 1 and pending_epi:
                        for args in pending_epi:
                            emit_epilogue(*args)
                        pending_epi = []
                pending_epi = [(ps_pA, icA), (ps_pB, icB)]
            for args in pending_epi:
                emit_epilogue(*args)
    return nc
